# revision 1
# baseline (speedup 1.0000x reference)
"""Trainium2 Bass kernel for nn_ContrastiveLoss (circle-loss contrastive).

Math (see reference):
    scores = im @ s.T                       [B, B], B=4096, D=1024
    lse_p[i] = logsumexp_j(256*(scores[i,j] - diag[i]))   (row LSE)
    lse_n[i] = logsumexp_j(256*(scores[j,i] - diag[i]))   (col LSE)
    out = softplus(lse(softplus(lse_p)/256)) + softplus(lse(softplus(lse_n)/256))

Device strategy: 4x2 core grid over (rows, cols) of the score matrix. Each
core computes its [1024, 2048] block with f32r matmuls (full-rate PE,
near-fp32 precision; operands pre-transposed on host) and reduces it to
logsumexp partials, spread across all five engines:

 per [128, 512] tile (PSUM, fp32):
  - row pass: DVE reduce_max -> ACT Exp(scale=256, bias=-256*max) with
    fused accum_out row sums (exact fp32 path)
  - DVE copies the tile to SBUF as bf16 (raw); Pool partition-reduces it
    to a [1, 512] partial column max which a gpsimd accumulate-DMA folds
    into the running half-chunk column max
 per half-chunk (4 row groups x 512 cols):
  - Pool broadcasts the finished column max to [128, 512]; per tile the
    DVE subtracts it from raw (bf16), ACT exponentiates, and a PE
    ones-matmul accumulates column sums in PSUM across the 4 row groups
 the very last half-chunk instead uses per-tile PE 128x128 transposes +
 segmented DVE col max + per-sub-tile ACT exp with fused accum, which has
 no cross-tile chain and therefore a short kernel tail.

The phase-2 units are pumped through a slot queue so they interleave with
later tiles' matmuls and the PE never waits on a column-pass chain.

Host combines the tiny (max, sumexp) partials with exact LSE algebra,
subtracts 256*diag, applies softplus, and finishes the scalar. The diagonal
stays inside the device sums: its term exp(256*(diag - max)) is numerically
zero unless diag ~= max, and such rows have minimal middle values, so the
effect on the final softplus-LSE is far below fp32 resolution. The column
pass sees bf16-rounded scores (~5e-4 final relative error); the row pass is
exact fp32.
"""

import numpy as np
from contextlib import ExitStack

import concourse.bass as bass
import concourse.bacc as bacc
import concourse.tile as tile
import concourse.mybir as mybir
from concourse.masks import make_identity

F32 = mybir.dt.float32
F32R = mybir.dt.float32r
BF16 = mybir.dt.bfloat16
AF = mybir.ActivationFunctionType
AX = mybir.AxisListType

B = 4096          # batch
D = 1024          # feature dim
GAMMA = 256.0
N_CORES = 8
GR, GC = 4, 2     # core grid: 4 row-shards x 2 col-shards
RB = B // GR      # rows per core   = 1024
CB = B // GC      # cols per core   = 2048
NM = RB // 128    # row groups per core  = 8
NN = CB // 512    # col chunks per core  = 4
NH = 2            # col-max halves per chunk (4 row groups each)
MH = NM // NH     # row groups per half   = 4
NK = D // 128     # contraction tiles     = 8
NT = NM * NN      # tiles per core        = 32

MM_DT = F32R      # matmul dtype: f32r = fp32 bits at bf16 PE rate
RAW_DT = BF16     # dtype of the copy used for the column pass


def _build():
    nc = bacc.Bacc("TRN2", target_bir_lowering=False, debug=False,
                   num_devices=N_CORES)
    imt = nc.dram_tensor("imt", [D, RB], MM_DT, kind="ExternalInput")
    st = nc.dram_tensor("st", [D, CB], MM_DT, kind="ExternalInput")
    rowm_d = nc.dram_tensor("rowm", [128, NT], F32, kind="ExternalOutput")
    rows_d = nc.dram_tensor("rows", [128, NT], F32, kind="ExternalOutput")
    colm_d = nc.dram_tensor("colm", [1, NN * NH * 512], BF16,
                            kind="ExternalOutput")
    cols_d = nc.dram_tensor("cols", [1, NN * NH * 512], F32,
                            kind="ExternalOutput")
    # last-half per-tile path: col partials per (m in 4..7, t in 0..3)
    colm4_d = nc.dram_tensor("colm4", [128, MH * 4], F32, kind="ExternalOutput")
    cols4_d = nc.dram_tensor("cols4", [128, MH * 4], F32, kind="ExternalOutput")

    with tile.TileContext(nc) as tc, ExitStack() as ctx:
        consts = ctx.enter_context(tc.tile_pool(name="consts", bufs=1))
        psA = ctx.enter_context(tc.tile_pool(name="psA", bufs=4, space="PSUM"))
        psC = ctx.enter_context(tc.tile_pool(name="psC", bufs=2, space="PSUM"))
        psB = ctx.enter_context(tc.tile_pool(name="psB", bufs=2, space="PSUM"))
        rawp = ctx.enter_context(tc.tile_pool(name="rawp", bufs=2))
        cmpp = ctx.enter_context(tc.tile_pool(name="cmpp", bufs=3))
        cmbp = ctx.enter_context(tc.tile_pool(name="cmbp", bufs=3))
        dp = ctx.enter_context(tc.tile_pool(name="dp", bufs=3))
        ep1 = ctx.enter_context(tc.tile_pool(name="ep1", bufs=2))
        smalls = ctx.enter_context(tc.tile_pool(name="smalls", bufs=4))

        ones = consts.tile([128, 1], RAW_DT)
        nc.gpsimd.memset(ones[:], 1.0)
        ident = consts.tile([128, 128], RAW_DT)
        make_identity(nc, ident[:])

        imt_sb = consts.tile([128, NK, RB], MM_DT)
        st_sb = consts.tile([128, NK, CB], MM_DT)
        rowm_sb = consts.tile([128, NT], F32)
        rows_sb = consts.tile([128, NT], F32)
        colm_sb = consts.tile([1, NN * NH * 512], BF16)
        cols_sb = consts.tile([1, NN * NH * 512], F32)
        colm4_sb = consts.tile([128, MH * 4], F32)
        cols4_sb = consts.tile([128, MH * 4], F32)

        # pre-warm the ACT Exp function table off the critical path
        warm = smalls.tile([128, 1], F32, tag="warm")
        nc.scalar.activation(warm[:], ones[:, 0:1], AF.Exp, bias=0.0, scale=0.0)

        imt_ap = imt.ap()
        st_ap = st.ap()

        def load_st(n, eng, ks=range(NK)):
            for k in ks:
                eng.dma_start(st_sb[:, k, 512 * n:512 * (n + 1)],
                              st_ap[128 * k:128 * (k + 1),
                                    512 * n:512 * (n + 1)])

        def load_imt(half, eng, ks=range(NK)):
            cols = slice(512 * half, 512 * (half + 1))
            for k in ks:
                eng.dma_start(imt_sb[:, k, cols],
                              imt_ap[128 * k:128 * (k + 1), cols])

        # Startup feed: st chunk 0 split across Pool/DVE SWDGE queues (fast),
        # imt half 0 on SP, then imt half 1 split SP/Pool, st chunk 1 on
        # Pool, chunk 2 on SP, chunk 3 on Pool at chunk-1 compute start.
        # ACT issues no DMAs - its FIFO would stall activations behind them.
        # 4 DMAs at the head of ACT's stream finish before its first exp is
        # needed (~4.5us in); everything else would stall activations.
        load_st(0, nc.gpsimd, range(0, NK, 2))
        load_st(0, nc.scalar, range(1, NK, 2))
        load_imt(0, nc.sync)
        load_imt(1, nc.sync, range(0, NK, 2))
        load_imt(1, nc.gpsimd, range(1, NK, 2))
        load_st(1, nc.gpsimd)
        load_st(2, nc.sync)

        # per-chunk / per-half state
        raw_chunk = [None] * NN    # [128, NM, 512] bf16
        cmw = {}                   # (n, h) -> [128, 512] bf16 partial maxes
        cmb = {}                   # (n, h) -> [128, 512] bf16 bcast col max
        psum_c = {}                # (n, h) -> [1, 512] f32 col sums

        def cidx(n, h):
            return (n * NH + h) * 512

        def is_tail_half(n, h):
            return n == NN - 1 and h == NH - 1

        def phase1_tile(n, m):
            """matmul tile + row stats + bf16 copy (+ col-max partial)."""
            idx = m * NN + n
            h = m // MH
            ps_a = psA.tile([128, 512], F32, tag="psA")
            for k in range(NK):
                nc.tensor.matmul(
                    ps_a[:],
                    imt_sb[:, k, 128 * m:128 * (m + 1)],
                    st_sb[:, k, 512 * n:512 * (n + 1)],
                    start=(k == 0),
                    stop=(k == NK - 1),
                )
            nc.vector.reduce_max(rowm_sb[:, idx:idx + 1], ps_a[:], axis=AX.X)
            nrm = smalls.tile([128, 1], F32, tag="nrm")
            nc.vector.tensor_scalar_mul(nrm[:], rowm_sb[:, idx:idx + 1], -GAMMA)
            e1 = ep1.tile([128, 512], BF16, tag="e1")
            nc.scalar.activation(e1[:], ps_a[:], AF.Exp, bias=nrm[:],
                                 scale=GAMMA, accum_out=rows_sb[:, idx:idx + 1])
            nc.vector.tensor_copy(raw_chunk[n][:, m, :], ps_a[:])
            if is_tail_half(n, h):
                return
            if m % MH == 0:
                cmw[(n, h)] = cmpp.tile([128, 512], BF16, tag="cmw",
                                        name=f"cmw{n}_{h}")
                nc.gpsimd.memset(cmw[(n, h)][:], -60000.0)
            # per-tile partial col max at a 32-aligned partition offset; one
            # more partition-reduce in phase2a folds the 4 partials together
            p0 = 32 * (m % MH)
            nc.gpsimd.reduce_max(cmw[(n, h)][p0:p0 + 1, :],
                                 raw_chunk[n][:, m, :], axis=AX.C)

        def phase2a(n, h):
            """combine + broadcast the half-chunk col max; ship it out."""
            dst = colm_sb[0:1, cidx(n, h):cidx(n, h) + 512]
            nc.gpsimd.reduce_max(dst, cmw[(n, h)][:], axis=AX.C)
            t = cmbp.tile([128, 512], BF16, tag="cmb", name=f"cmb{n}_{h}")
            cmb[(n, h)] = t
            nc.gpsimd.partition_broadcast(t[:], dst)
            nc.sync.dma_start(colm_d.ap()[0:1, cidx(n, h):cidx(n, h) + 512],
                              dst)

        def phase2b(n, h, m):
            """col-sum contribution of row group m (in half h) of chunk n."""
            d = dp.tile([128, 512], BF16, tag="d")
            nc.vector.tensor_sub(d[:], raw_chunk[n][:, m, :], cmb[(n, h)][:])
            e2 = dp.tile([128, 512], BF16, tag="e2")
            nc.scalar.activation(e2[:], d[:], AF.Exp, bias=0.0, scale=GAMMA)
            nc.tensor.matmul(psum_c[(n, h)][:], ones[:], e2[:],
                             start=(m % MH == 0), stop=(m % MH == MH - 1))

        def phase2c(n, h):
            nc.vector.tensor_copy(cols_sb[0:1, cidx(n, h):cidx(n, h) + 512],
                                  psum_c[(n, h)][:])
            nc.sync.dma_start(cols_d.ap()[0:1, cidx(n, h):cidx(n, h) + 512],
                              cols_sb[0:1, cidx(n, h):cidx(n, h) + 512])

        def tail_tile(n, m):
            """self-contained col pass for one tile of the final half."""
            j = m - MH * (NH - 1)
            ps_b = psB.tile([128, 4, 128], RAW_DT, tag="psB")
            for t in range(4):
                nc.tensor.transpose(ps_b[:, t, :],
                                    raw_chunk[n][:, m, 128 * t:128 * (t + 1)],
                                    ident[:])
            nc.vector.reduce_max(colm4_sb[:, 4 * j:4 * j + 4], ps_b[:, :, :],
                                 axis=AX.X)
            ncm = smalls.tile([128, 4], F32, tag="ncm")
            nc.vector.tensor_scalar_mul(ncm[:], colm4_sb[:, 4 * j:4 * j + 4],
                                        -GAMMA)
            e4 = ep1.tile([128, 4, 128], BF16, tag="e4")
            for t in range(4):
                nc.scalar.activation(e4[:, t, :], ps_b[:, t, :], AF.Exp,
                                     bias=ncm[:, t:t + 1], scale=GAMMA)
            # one segmented DVE sum replaces four ACT accumulator reads
            nc.vector.reduce_sum(cols4_sb[:, 4 * j:4 * j + 4], e4[:, :, :],
                                 axis=AX.X)

        pending = []   # entries: (ready_slot, thunk)
        slot = [0]

        def pump():
            slot[0] += 1
            # 10 units are enqueued per 8 slots; drain 2 when backed up
            k = 2 if len(pending) > 3 else 1
            for _ in range(k):
                if pending and pending[0][0] <= slot[0]:
                    pending.pop(0)[1]()

        for n in range(NN):
            if n == 1:
                load_st(3, nc.gpsimd)
            raw_chunk[n] = rawp.tile([128, NM, 512], RAW_DT, tag="raw",
                                     name=f"raw{n}")
            for m in range(NM):
                phase1_tile(n, m)
                h = m // MH
                if is_tail_half(n, h):
                    pending.append(
                        (slot[0] + 1, lambda n_=n, m_=m: tail_tile(n_, m_)))
                pump()
                if m % MH == MH - 1 and not is_tail_half(n, h):
                    psum_c[(n, h)] = psC.tile([1, 512], F32, tag="psC",
                                              name=f"psc{n}_{h}")
                    phase2a(n, h)
                    # let the col-max chain land before the PE meets the
                    # first ones-matmul
                    ready = slot[0] + 3
                    for mm_ in range(MH * h, MH * (h + 1)):
                        pending.append(
                            (ready,
                             lambda n_=n, h_=h, m_=mm_: phase2b(n_, h_, m_)))
                    pending.append((ready, lambda n_=n, h_=h: phase2c(n_, h_)))
        while pending:
            slot[0] += 10
            pump()

        nc.sync.dma_start(rowm_d.ap(), rowm_sb[:])
        nc.sync.dma_start(rows_d.ap(), rows_sb[:])
        nc.sync.dma_start(colm4_d.ap(), colm4_sb[:])
        nc.sync.dma_start(cols4_d.ap(), cols4_sb[:])

    nc.compile()
    return nc


_NC = None


def _get_nc():
    global _NC
    if _NC is None:
        _NC = _build()
    return _NC


def make_in_maps(im, s):
    im = np.asarray(im, dtype=np.float32)
    s = np.asarray(s, dtype=np.float32)
    im_t = np.ascontiguousarray(im.T)   # [D, B]
    s_t = np.ascontiguousarray(s.T)     # [D, B]
    in_maps = []
    for c in range(N_CORES):
        a, b = divmod(c, GC)
        in_maps.append({
            "imt": np.ascontiguousarray(im_t[:, a * RB:(a + 1) * RB]),
            "st": np.ascontiguousarray(s_t[:, b * CB:(b + 1) * CB]),
        })
    return in_maps


def host_combine(results, im, s):
    """Combine per-core (max, sumexp) partials into the final scalar."""
    im = np.asarray(im, dtype=np.float32)
    s = np.asarray(s, dtype=np.float32)
    diag = np.einsum("ij,ij->i", im.astype(np.float64), s.astype(np.float64))

    # row partials: global row r = a*RB + 128*m + p, one partial per (b, n)
    row_max = np.full((B, GC * NN), -np.inf)
    row_sum = np.zeros((B, GC * NN))
    # col partials: up to GR * (NH + MH) slots per column
    PC = GR * (NH + MH)
    col_max = np.full((B, PC), -np.inf)
    col_sum = np.zeros((B, PC))

    for c in range(N_CORES):
        a, b = divmod(c, GC)
        rowm = np.asarray(results[c]["rowm"], dtype=np.float64)
        rows_ = np.asarray(results[c]["rows"], dtype=np.float64)
        colm = np.asarray(results[c]["colm"]).astype(np.float64)[0]
        cols_ = np.asarray(results[c]["cols"], dtype=np.float64)[0]
        colm4 = np.asarray(results[c]["colm4"], dtype=np.float64)
        cols4 = np.asarray(results[c]["cols4"], dtype=np.float64)
        for m in range(NM):
            r = a * RB + 128 * m + np.arange(128)
            for n in range(NN):
                idx = m * NN + n
                row_max[r, b * NN + n] = rowm[:, idx]
                row_sum[r, b * NN + n] = rows_[:, idx]
        for n in range(NN):
            for h in range(NH):
                if n == NN - 1 and h == NH - 1:
                    continue
                j = b * CB + 512 * n + np.arange(512)
                w = (n * NH + h) * 512
                col_max[j, a * NH + h] = colm[w:w + 512]
                col_sum[j, a * NH + h] = cols_[w:w + 512]
        # final half of the last chunk: per (row-group, sub-tile) partials
        for jm in range(MH):
            for t in range(4):
                j = b * CB + 512 * (NN - 1) + 128 * t + np.arange(128)
                w = 4 * jm + t
                col_max[j, GR * NH + a * MH + jm] = colm4[:, w]
                col_sum[j, GR * NH + a * MH + jm] = cols4[:, w]

    def combine_lse(pmax, psum):
        m256 = GAMMA * pmax
        mm = m256.max(axis=1, keepdims=True)
        s_ = np.sum(psum * np.exp(np.clip(m256 - mm, -745.0, 0.0)), axis=1)
        return mm[:, 0] + np.log(s_)

    lse_row = combine_lse(row_max, row_sum)
    lse_col = combine_lse(col_max, col_sum)

    def softplus(x):
        return np.logaddexp(0.0, x)

    middle1 = softplus(lse_row - GAMMA * diag) / GAMMA   # cost_s (rows)
    middle = softplus(lse_col - GAMMA * diag) / GAMMA    # cost_im (cols)

    def lse_vec(v):
        m = v.max()
        return m + np.log(np.sum(np.exp(v - m)))

    out = softplus(lse_vec(middle1)) + softplus(lse_vec(middle))
    return np.asarray(out, dtype=np.float32)


def kernel(im, s):
    from concourse.bass_utils import run_bass_kernel_spmd
    nc = _get_nc()
    in_maps = make_in_maps(im, s)
    res = run_bass_kernel_spmd(nc, in_maps, core_ids=list(range(N_CORES)))
    return host_combine(res.results, im, s)



# revision 2
# speedup vs baseline: 2.8109x; 2.8109x over previous
"""Trainium2 Bass kernel for nn_ContrastiveLoss (circle-loss contrastive).

Math (see reference):
    scores = im @ s.T                       [B, B], B=4096, D=1024
    out = softplus(lse(softplus(256*(rowlse - diag))/256))
        + softplus(lse(softplus(256*(collse - diag))/256))

With gamma=256 every logsumexp is max-dominated: replacing each row/col
LSE by the row/col max changes the final scalar by < 1e-9 relative (the
nearest competitor of a row max is typically several units below it, and
exp(-256 * gap) vanishes).  The kernel therefore only computes the score
matrix and its row maxes on device, ships bf16 copies of the scores, and
the host finishes maxes + softplus/LSE algebra in numpy at f64.

Device strategy: 4x2 core grid over (rows, cols) of the score matrix;
each core computes a [1024, 2048] block as 32 [128, 512] PSUM tiles with
fp8-e4m3 DoubleRow matmuls (2 k-tiles of 128 per instruction, 2x PE
rate; measured end-to-end quantization error 2.8e-3 vs the 2e-2 gate).
Per tile a single DVE tensor_tensor_reduce computes the exact per-row
max (accum_out, f32) and simultaneously writes a bf16 copy of the tile
to SBUF (out = max(psum, -inf)).  Each chunk's bf16 copy block is DMA'd
to HBM as soon as its 8 tiles finish; the host reduces those for column
maxes.  No column-max machinery on device at all.
"""

import numpy as np
from contextlib import ExitStack

import concourse.bass as bass
import concourse.bacc as bacc
import concourse.tile as tile
import concourse.mybir as mybir

F32 = mybir.dt.float32
BF16 = mybir.dt.bfloat16
FP8 = mybir.dt.float8e4
AX = mybir.AxisListType
OP = mybir.AluOpType
DR = mybir.MatmulPerfMode.DoubleRow

B = 4096          # batch
D = 1024          # feature dim
GAMMA = 256.0
N_CORES = 8
GR, GC = 4, 2     # core grid: 4 row-shards x 2 col-shards
RB = B // GR      # rows per core   = 1024
CB = B // GC      # cols per core   = 2048
NM = RB // 128    # row groups per core  = 8
NN = CB // 512    # col chunks per core  = 4
NK = D // 128     # 128-deep k-tiles     = 8
NKP = NK // 2     # DoubleRow k-pairs    = 4
NT = NM * NN      # tiles per core       = 32


def _build():
    nc = bacc.Bacc("TRN2", target_bir_lowering=False, debug=False,
                   num_devices=N_CORES)
    imt = nc.dram_tensor("imt", [NK, 128, RB], FP8, kind="ExternalInput")
    st = nc.dram_tensor("st", [NK, 128, CB], FP8, kind="ExternalInput")
    rowm_d = nc.dram_tensor("rowm", [128, NT], F32, kind="ExternalOutput")
    raw_d = nc.dram_tensor("raw", [NN, 128, NM, 512], BF16,
                           kind="ExternalOutput")

    with tile.TileContext(nc) as tc, ExitStack() as ctx:
        consts = ctx.enter_context(tc.tile_pool(name="consts", bufs=1))
        psA = ctx.enter_context(tc.tile_pool(name="psA", bufs=8, space="PSUM"))
        rawp = ctx.enter_context(tc.tile_pool(name="rawp", bufs=2))

        imt_sb = consts.tile([128, NK, RB], FP8)
        st_sb = consts.tile([128, NK, CB], FP8)
        rowm_sb = consts.tile([128, NT], F32)
        neg = consts.tile([128, 512], BF16)
        nc.gpsimd.memset(neg[:], -60000.0)

        # Input feed, ordered so tile (0, 0) can start ASAP and each later
        # tile's operands land before the PE reaches it.
        imt_ap = imt.ap()
        st_ap = st.ap()

        def load_st(n):
            nc.sync.dma_start(
                st_sb[:, :, 512 * n:512 * (n + 1)],
                st_ap[:, :, 512 * n:512 * (n + 1)].rearrange("k p c -> p k c"))

        def load_imt(m):
            nc.sync.dma_start(
                imt_sb[:, :, 128 * m:128 * (m + 1)],
                imt_ap[:, :, 128 * m:128 * (m + 1)].rearrange("k p c -> p k c"))

        load_st(0)
        for m in range(NM):
            load_imt(m)
        for n in range(1, NN):
            load_st(n)

        for n in range(NN):
            raw_n = rawp.tile([128, NM, 512], BF16, tag="raw", name=f"raw{n}")
            for m in range(NM):
                ps = psA.tile([128, 512], F32, tag="ps")
                for q in range(NKP):
                    nc.tensor.matmul(
                        ps[:],
                        imt_sb[:, 2 * q:2 * q + 2, 128 * m:128 * (m + 1)],
                        st_sb[:, 2 * q:2 * q + 2, 512 * n:512 * (n + 1)],
                        start=(q == 0),
                        stop=(q == NKP - 1),
                        perf_mode=DR,
                    )
                # one DVE op: bf16 copy of the tile + exact f32 row max
                nc.vector.tensor_tensor_reduce(
                    raw_n[:, m, :], ps[:], neg[:], 1.0, -1.0e30,
                    OP.max, OP.max, rowm_sb[:, n * NM + m:n * NM + m + 1])
            nc.sync.dma_start(raw_d.ap()[n], raw_n[:])
        nc.sync.dma_start(rowm_d.ap(), rowm_sb[:])

    nc.compile()
    return nc


_NC = None


def _get_nc():
    global _NC
    if _NC is None:
        _NC = _build()
    return _NC


def make_in_maps(im, s):
    import ml_dtypes
    im8 = np.ascontiguousarray(np.asarray(im, dtype=np.float32).T) \
        .astype(ml_dtypes.float8_e4m3)              # [D, B]
    s8 = np.ascontiguousarray(np.asarray(s, dtype=np.float32).T) \
        .astype(ml_dtypes.float8_e4m3)              # [D, B]
    in_maps = []
    for c in range(N_CORES):
        a, b = divmod(c, GC)
        in_maps.append({
            "imt": np.ascontiguousarray(
                im8[:, a * RB:(a + 1) * RB].reshape(NK, 128, RB)),
            "st": np.ascontiguousarray(
                s8[:, b * CB:(b + 1) * CB].reshape(NK, 128, CB)),
        })
    return in_maps


def _bf16_to_f32(x):
    u = np.ascontiguousarray(x).view(np.uint16).astype(np.uint32) << np.uint32(16)
    return u.view(np.float32)


def host_combine(results, im, s):
    """row/col maxes -> softplus/LSE algebra at f64."""
    im = np.asarray(im, dtype=np.float32)
    s = np.asarray(s, dtype=np.float32)
    diag = np.einsum("ij,ij->i", im.astype(np.float64), s.astype(np.float64))

    rm = np.full(B, -np.inf)
    cm = np.full(B, -np.inf)
    for c in range(N_CORES):
        a, b = divmod(c, GC)
        rowm = np.asarray(results[c]["rowm"], dtype=np.float64)  # [128, NT]
        # idx = n*NM + m ; row r = a*RB + 128*m + p
        rmc = rowm.reshape(128, NN, NM).max(axis=1)              # [128, NM]
        r0 = a * RB
        rm_view = rm[r0:r0 + RB].reshape(NM, 128)                # [m, p]
        np.maximum(rm_view, rmc.T, out=rm_view)
        raw = _bf16_to_f32(np.asarray(results[c]["raw"]))        # [NN,128,NM,512]
        cmc = raw.max(axis=(1, 2))                               # [NN, 512]
        c0 = b * CB
        cm_view = cm[c0:c0 + CB].reshape(NN, 512)
        np.maximum(cm_view, cmc.astype(np.float64), out=cm_view)

    def sp(v):
        return np.logaddexp(0.0, v)

    def lse(v):
        mx = v.max()
        return mx + np.log(np.sum(np.exp(v - mx)))

    mid1 = sp(GAMMA * (rm - diag)) / GAMMA   # caption-contrastive rows
    mid = sp(GAMMA * (cm - diag)) / GAMMA    # image-contrastive cols
    out = sp(lse(mid1)) + sp(lse(mid))
    return np.asarray(out, dtype=np.float32)


def kernel(im, s):
    from concourse.bass_utils import run_bass_kernel_spmd
    nc = _get_nc()
    in_maps = make_in_maps(im, s)
    res = run_bass_kernel_spmd(nc, in_maps, core_ids=list(range(N_CORES)))
    return host_combine(res.results, im, s)
